# revision 24
# baseline (speedup 1.0000x reference)
"""Distributed multi-head attention kernel for one TRN2 chip (8 NeuronCores).

Problem: B=2, T=2048, D=1024, H=16 heads (hd=64).
  qkv = x @ w_attn + b_attn ; per-head softmax((q k^T)/sqrt(hd) + 2*mask) v
  out = attn @ w_proj + b_proj

Sharding: tensor-parallel over heads. Core c owns heads {2c, 2c+1}.
  - Q/K projections in transposed layout (QT/KT: [hd, T]); head h lives at
    partitions [64h, 64h+64) of qt2/kt2.
  - S^T[kv, q] = K_h^T Q_h as K=64 row-tiled matmul pairs: the two heads
    occupy PE row-halves (tile_position (0,0)/(64,0)) and stream
    concurrently - 2x over the zero-padded K=128 formulation.
  - V is computed directly in natural [t, hd] layout (lhsT = x chunk,
    rhs = w_v), eliminating PE transposes.
  - Softmax denominator comes from a ones-column appended to V in the PV
    matmul (O_ext = [V|1]^T @ P^T); PV is stream-bound and runs at the
    N-cycle roofline already.
  - Exp runs on ScalarE only (it is the pacing engine); all PSUM->SBUF
    copies are on VectorE.
  - Output rows are STRIPED across cores in 64-row chunks (global 64-row
    chunk R -> core R%8), so every (b, qb) q-block completes a full
    AllToAll payload for all 8 destination cores.  8 mini-AllToAlls
    ([8,128,64] bf16 each) fire as soon as each q-block normalizes; all
    but the last are fully hidden under the chunk stream, killing the
    ~50us serial tail of the 2-collective design.
  - The final projection consumes pin pairs (two mini-A2A outputs = 128
    rows, M=128 lhsT) and is scheduled into PE slack: pairs 0/1 in the
    ScalarE-bound late stream, pair 2 inside the last collective's
    flight window, pair 3 right after it lands.
  - All matmul operands bf16; accumulation + softmax statistics fp32.
  - PSUM: 2x[128,1024] S^T/exp ping-pong (4 banks) + 2x[128,512] aux
    (QKV/proj/warmup, 2 banks) + 2x[65,512] PV accumulators (2 banks).
"""

import sys

sys.path.insert(0, "/opt/trn_rl_repo")

import numpy as np

B, T, D = 2, 2048, 1024
H = 16
HD = D // H
NCORES = 8
HPC = H // NCORES          # heads per core = 2
BT = B * T                 # 4096 global rows
ROWS_PER_CORE = BT // NCORES   # 512
TB = 512                   # t-block width for QKV projection
NTB = BT // TB             # 8
NKD = D // 128             # 8 contraction chunks over D
QB = 512                   # q-block width in attention
NQB = T // QB              # 4 per (batch, head)
NKV = T // 128             # 16 kv chunks per batch
PVLAG = 6                  # PV trails exp by this many kv chunks
NBLK = B * NQB             # 8 q-blocks == 8 mini collectives

_CACHE = {}
import ml_dtypes
BF16 = ml_dtypes.bfloat16


def _build(with_mask: bool, with_battn: bool, with_bproj: bool):
    import concourse.bass as bass
    import concourse.tile as tile
    from concourse import bacc, mybir

    f32 = mybir.dt.float32
    bf16 = mybir.dt.bfloat16
    Exp = mybir.ActivationFunctionType.Exp

    nc = bacc.Bacc("TRN2", target_bir_lowering=False, debug=False,
                   num_devices=NCORES)
    rg = [list(range(NCORES))]

    xt = nc.dram_tensor("xt", [D, BT], bf16, kind="ExternalInput")
    w_qk = nc.dram_tensor("w_qk", [D, 256], bf16, kind="ExternalInput")
    w_v = nc.dram_tensor("w_v", [D, 128], bf16, kind="ExternalInput")
    w_proj = nc.dram_tensor("w_proj", [D, D], bf16, kind="ExternalInput")
    if with_mask:
        mask2 = nc.dram_tensor("mask2", [128, B * NKV], f32,
                               kind="ExternalInput")
    if with_battn:
        b_qk = nc.dram_tensor("b_qk", [1, 256], bf16, kind="ExternalInput")
        b_v = nc.dram_tensor("b_v", [1, 128], bf16, kind="ExternalInput")
    if with_bproj:
        b_proj = nc.dram_tensor("b_proj", [1, D], bf16, kind="ExternalInput")
    out = nc.dram_tensor("out", [ROWS_PER_CORE, D], f32, kind="ExternalOutput")

    with tile.TileContext(nc, num_cores=NCORES) as tc:
        from contextlib import ExitStack
        with ExitStack() as ctx:
            const = ctx.enter_context(tc.tile_pool(name="const", bufs=1))
            xt_pool = ctx.enter_context(tc.tile_pool(name="xtp", bufs=4))
            qk_pool = ctx.enter_context(tc.tile_pool(name="qkp", bufs=1))
            pt_pool = ctx.enter_context(tc.tile_pool(name="ptp", bufs=9))
            lbc_pool = ctx.enter_context(tc.tile_pool(name="lbc", bufs=2))
            pin_pool = ctx.enter_context(tc.tile_pool(name="pin", bufs=56))
            out_pool = ctx.enter_context(tc.tile_pool(name="outp", bufs=2))
            # PSUM: hot = S^T/exp ping-pong, 2 slots x [128,1024]f32
            # (2 banks each); aux = QKV/proj/warmup accumulators, 2 slots
            # x [128,512]f32 (1 bank); acc = PV accumulators 2 x [65,512].
            hot = ctx.enter_context(tc.tile_pool(name="hot", bufs=2,
                                                 space="PSUM"))
            aux = ctx.enter_context(tc.tile_pool(name="aux", bufs=2,
                                                 space="PSUM"))
            acc_pool = ctx.enter_context(tc.tile_pool(name="accp", bufs=2,
                                                      space="PSUM"))
            dram = ctx.enter_context(tc.tile_pool(name="dram", bufs=4,
                                                  space="DRAM"))

            # ---- ACT exp-table preload (runs at t~0 on the scalar queue) --
            dmy = const.tile([1, 8], bf16)
            nc.vector.memset(dmy[:], 0.0)
            dmy2 = const.tile([1, 8], f32)
            nc.scalar.activation(out=dmy2[:], in_=dmy[:], func=Exp)

            # ---- PE warmup: dependency-free matmuls run during the input
            # DMA wait, flipping the HAM clock gate to full rate before the
            # first real QKV matmuls issue.
            wrm = const.tile([128, TB], bf16)
            nc.vector.memset(wrm[:], 0.0)
            wps = [aux.tile([128, TB], f32, tag="aux", name="wps")
                   for _ in range(2)]
            for i in range(10):
                nc.tensor.matmul(
                    wps[i % 2][:],
                    lhsT=wrm[:, 0:128], rhs=wrm[:],
                    start=True, stop=True)

            # ---- constants ----
            # w_qk packed per D-chunk: [128, NKD, 256]; group g cols
            # [128g, 128g+128): g0=[q_h0/8|k_h0] g1=[q_h1/8|k_h1]
            wqk_sb = const.tile([128, NKD, 256], bf16)
            for half in range(2):
                nc.sync.dma_start(
                    out=wqk_sb[:, 4 * half:4 * (half + 1), :],
                    in_=w_qk[:].rearrange("(a p) c -> p a c", p=128)[
                        :, 4 * half:4 * (half + 1), :])
            # w_v packed per D-chunk: [128, NKD, 128]; cols [v_h0|v_h1]
            wv_sb = const.tile([128, NKD, 128], bf16)
            nc.scalar.dma_start(
                out=wv_sb[:],
                in_=w_v[:].rearrange("(a p) c -> p a c", p=128))
            wproj_sb = const.tile([128, NKD, D], bf16)

            def emit_wproj_loads():
                for half in range(2):
                    nc.sync.dma_start(
                        out=wproj_sb[:, 4 * half:4 * (half + 1), :],
                        in_=w_proj[:].rearrange("(a p) c -> p a c", p=128)[
                            :, 4 * half:4 * (half + 1), :])

            if with_mask:
                # mask (already doubled on host): [128, B, NKV]
                mask_sb = const.tile([128, B, NKV], f32)
                nc.sync.dma_start(out=mask_sb[:],
                                  in_=mask2[:].rearrange("p (b j) -> p b j",
                                                         b=B))
            if with_battn:
                bqk_sb = const.tile([1, 256], bf16)
                nc.sync.dma_start(out=bqk_sb[:], in_=b_qk[:])
                bv_sb = const.tile([1, 128], bf16)
                nc.sync.dma_start(out=bv_sb[:], in_=b_v[:])
                ones_row = const.tile([1, TB], bf16)
                nc.vector.memset(ones_row[:], 1.0)
            if with_bproj:
                bproj_sb = const.tile([1, D], bf16)
                nc.sync.dma_start(out=bproj_sb[:], in_=b_proj[:])
                ones_col = const.tile([1, 128], bf16)
                nc.vector.memset(ones_col[:], 1.0)

            ones64 = const.tile([1, 64], bf16)
            nc.vector.memset(ones64[:], 1.0)

            # persistent activations. Head h at partitions [64h, 64h+64).
            qt2 = qk_pool.tile([128, BT], bf16, tag="qt2", name="qt2")
            kt2 = qk_pool.tile([128, BT], bf16, tag="kt2", name="kt2")
            ot = qk_pool.tile([128, BT], bf16, tag="ot", name="ot")
            # V natural layout + ones column: subtile s = 32b + 2j + h is
            # [128 kv, 65] = [V_chunk | 1].
            vnat = qk_pool.tile([128, 2 * NKV * HPC, 65], bf16, tag="vnat",
                                name="vnat")
            nc.vector.memset(vnat[:, :, 64:65], 1.0)

            # ---- QKV projection ----
            # q/k: transposed outputs via stationary weights.
            # v: natural output via stationary x chunks.
            def qkv_tblock_units(tb):
                xt_t = xt_pool.tile([128, NKD, TB], bf16, tag="xt", name="xt")

                def dma_unit():
                    xsrc = xt[:].rearrange("(a p) t -> p a t", p=128)
                    for half in range(2):
                        nc.sync.dma_start(
                            out=xt_t[:, 4 * half:4 * (half + 1), :],
                            in_=xsrc[:, 4 * half:4 * (half + 1),
                                     TB * tb:TB * (tb + 1)])

                # finer-grained halves so PE-queue fillers never stall
                # the scalar engine for more than ~1us
                qkps = {}

                def qk_half(g, half):
                    if g not in qkps:
                        qkps[g] = aux.tile([128, TB], f32, tag="aux",
                                           name="qkps")
                    ps = qkps[g]
                    for d in range(4 * half, 4 * half + 4):
                        nc.tensor.matmul(
                            ps[:],
                            lhsT=wqk_sb[:, d, 128 * g:128 * (g + 1)],
                            rhs=xt_t[:, d, :],
                            start=(d == 0),
                            stop=(d == NKD - 1) and not with_battn)
                    if half == 0:
                        return
                    if with_battn:
                        nc.tensor.matmul(
                            ps[:],
                            lhsT=bqk_sb[:, 128 * g:128 * (g + 1)],
                            rhs=ones_row[:],
                            start=False, stop=True)
                    cs = TB * tb
                    nc.vector.tensor_copy(
                        out=qt2[64 * g:64 * (g + 1), cs:cs + TB],
                        in_=ps[0:64, :])
                    nc.vector.tensor_copy(
                        out=kt2[64 * g:64 * (g + 1), cs:cs + TB],
                        in_=ps[64:128, :])
                    del qkps[g]

                def v_chunk(c4):
                    ps = aux.tile([128, TB], f32, tag="aux", name="vps")
                    for d in range(NKD):
                        nc.tensor.matmul(
                            ps[:, 0:128],
                            lhsT=xt_t[:, d, 128 * c4:128 * (c4 + 1)],
                            rhs=wv_sb[:, d, :],
                            start=(d == 0),
                            stop=(d == NKD - 1) and not with_battn)
                    if with_battn:
                        nc.tensor.matmul(
                            ps[:, 0:128],
                            lhsT=ones_row[:, 0:128],
                            rhs=bv_sb[:],
                            start=False, stop=True)
                    gj = 4 * tb + c4          # global 128-row chunk id
                    b_, j = gj // NKV, gj % NKV
                    for h in range(HPC):
                        s = 32 * b_ + 2 * j + h
                        nc.vector.tensor_copy(
                            out=vnat[:, s, 0:64],
                            in_=ps[:, 64 * h:64 * (h + 1)])

                return [dma_unit,
                        lambda: qk_half(0, 0), lambda: qk_half(0, 1),
                        lambda: qk_half(1, 0), lambda: qk_half(1, 1),
                        lambda: v_chunk(0), lambda: v_chunk(1),
                        lambda: v_chunk(2), lambda: v_chunk(3)]

            # ---- attention: one global software-pipelined chunk stream ----
            # chunk g = (b, qb, j); S^T+exp at position g, PV at g+PVLAG.
            CHUNKS = [(b, qb, j) for b in range(B) for qb in range(NQB)
                      for j in range(NKV)]
            pts = {}
            ps_o_by_q = {}

            def emit_st(g):
                b, qb, j = CHUNKS[g]
                c0 = 2048 * b + QB * qb
                k0 = 2048 * b + 128 * j
                # both heads' S^T chunk in one 2-bank slot; the two K=64
                # row-tiled matmuls hit disjoint PE row groups and
                # disjoint banks -> concurrent
                st = hot.tile([128, 2 * QB], f32, tag="hot", name="st")
                for h in range(HPC):
                    nc.tensor.matmul(
                        st[:, QB * h:QB * (h + 1)],
                        lhsT=kt2[64 * h:64 * (h + 1), k0:k0 + 128],
                        rhs=qt2[64 * h:64 * (h + 1), c0:c0 + QB],
                        start=True, stop=True)
                pt = pt_pool.tile([128, 2 * QB], bf16, tag="pt", name="pt")
                if with_mask:
                    nc.scalar.activation(out=pt[:], in_=st[:], func=Exp,
                                         bias=mask_sb[:, b, j:j + 1],
                                         scale=1.0)
                else:
                    nc.scalar.activation(out=pt[:], in_=st[:], func=Exp)
                pts[g] = pt

            def emit_pv(g):
                b, qb, j = CHUNKS[g]
                if j == 0:
                    ps_o_by_q[(b, qb)] = [
                        acc_pool.tile([65, QB], f32, tag="acc", name="acc")
                        for _ in range(HPC)]
                ps_o = ps_o_by_q[(b, qb)]
                pt = pts.pop(g)
                for h in range(HPC):
                    s = 32 * b + 2 * j + h
                    nc.tensor.matmul(
                        ps_o[h][:],
                        lhsT=vnat[:, s, :],
                        rhs=pt[:, QB * h:QB * (h + 1)],
                        start=(j == 0), stop=(j == NKV - 1),
                        skip_group_check=True)
                if j == NKV - 1:
                    normalize(b, qb)

            def normalize(b, qb):
                # Normalization is DEFERRED past the AllToAll: copy the
                # UNNORMALIZED O^T straight to bf16 ot (freeing both PSUM
                # accumulator banks fast) and compute the bf16 reciprocal
                # rows; both ride the collective (130-row slots) and the
                # scale is applied to the pins after the A2A.  This keeps
                # the normalize -> collective-trigger chain to ~3us.
                c0 = 2048 * b + QB * qb
                ps_o = ps_o_by_q.pop((b, qb))
                lsbs = []
                for h in range(HPC):
                    nc.vector.tensor_copy(
                        out=ot[64 * h:64 * (h + 1), c0:c0 + QB],
                        in_=ps_o[h][0:64, :])
                    lsb = lbc_pool.tile([1, QB], f32, tag="lsb", name="lsb")
                    nc.vector.tensor_copy(out=lsb[:], in_=ps_o[h][64:65, :])
                    lsbs.append(lsb)
                lrecs = []
                for h in range(HPC):
                    lrec32 = lbc_pool.tile([1, QB], f32, tag="lrec32",
                                           name="lrec32")
                    nc.vector.reciprocal_approx_fast(
                        out=lrec32[:], in_=lsbs[h][:])
                    lrec = lbc_pool.tile([1, QB], bf16, tag="lrec",
                                         name="lrec")
                    nc.vector.tensor_copy(out=lrec[:], in_=lrec32[:])
                    lrecs.append(lrec)
                a2a_block(b, qb, lrecs)

            # ---- striped mini-AllToAll (head-space -> row-space) ----
            # q-block (b, qb) == block k = 4b + qb.  Dest core j receives
            # ot cols [2048b + 512qb + 64j, +64) from every core and owns
            # local out rows [64k, 64k+64).  Blocks are shipped in PAIRS
            # (one collective per 128 output rows): slot j = [128, 2, 64].
            from concourse import mybir as _mb

            def prime_cc():
                # tiny dummy AllToAll (garbage data, output unused) so the
                # CC stream's one-time barrier + init (~30-45us) runs
                # during the PE-bound QKV phase
                pin = dram.tile([NCORES, 1, 64], bf16, tag="a2ain",
                                name="prime_in")
                pout = dram.tile([NCORES, 1, 64], bf16, tag="a2aout",
                                 name="prime_out")
                nc.gpsimd.collective_compute(
                    "AllToAll", _mb.AluOpType.bypass, replica_groups=rg,
                    ins=[pin.opt()], outs=[pout.opt()])

            a_ins = {}
            a2a_outs = {}

            def a2a_block(b, qb, lrecs):
                # blocks ship in pairs: collective c covers blocks (2c,
                # 2c+1) == output rows [128c, 128c+128).  Slot rows 128/129
                # carry the bf16 reciprocal softmax denominators of this
                # core's two heads for the slot's 64 t positions.
                k = NQB * b + qb
                c, idx = k // 2, k % 2
                if idx == 0:
                    a_ins[c] = dram.tile([NCORES, 130, 2, 64], bf16,
                                         tag="a2ain", name="a2ain")
                a_in = a_ins[c]
                base = 2048 * b + QB * qb
                for j in range(NCORES):
                    nc.sync.dma_start(
                        out=a_in[j][0:128, idx, :],
                        in_=ot[:, base + 64 * j:base + 64 * (j + 1)])
                    for h in range(HPC):
                        nc.sync.dma_start(
                            out=a_in[j][128 + h:129 + h, idx, :],
                            in_=lrecs[h][:, 64 * j:64 * (j + 1)])
                if idx == 0:
                    return
                del a_ins[c]
                a_out = dram.tile([NCORES, 130, 2, 64], bf16, tag="a2aout",
                                  name="a2aout")
                nc.gpsimd.collective_compute(
                    "AllToAll", _mb.AluOpType.bypass, replica_groups=rg,
                    ins=[a_in.opt()], outs=[a_out.opt()])
                a2a_outs[c] = a_out

            # pins: collective c covers out rows [128c, 128c+128); tile j
            # is the [128, 128] lhsT chunk for contraction block j, scaled
            # by the deferred softmax denominators after the A2A.
            pins_by_c = {}

            def pins_load(c):
                # on the gpsimd queue so the wait-for-collective does not
                # head-of-line-block the sync queue's normalize DMAs
                a_out = a2a_outs.pop(c)
                pins_by_c[c] = []
                for j in range(NCORES):
                    raw = pin_pool.tile([128, 2, 64], bf16, tag="pin",
                                        name="pin")
                    nc.gpsimd.dma_start(out=raw[:], in_=a_out[j][0:128])
                    sc = pin_pool.tile([128, 2, 64], bf16, tag="pin",
                                       name="psc")
                    for h in range(HPC):
                        nc.gpsimd.dma_start(
                            out=sc[64 * h:64 * (h + 1), :, :],
                            in_=a_out[j][128 + h:129 + h, :, :]
                            .to_broadcast([64, 2, 64]))
                    # multiply on GpSimd: it is the one queue that already
                    # blocks on collectives, so a late flight can never
                    # head-of-line-block VectorE/ScalarE/PE work
                    pin = pin_pool.tile([128, 2, 64], bf16, tag="pin",
                                        name="pins")
                    nc.gpsimd.tensor_mul(out=pin[:], in0=raw[:], in1=sc[:])
                    pins_by_c[c].append(pin)

            proj_state = {}

            def proj_half(c, n, uh):
                # collective c -> out rows [128c, 128c+128), cols [512n,
                # +512), contraction blocks j in [4uh, 4uh+4)
                pins = pins_by_c[c]
                mc, r0 = 128, 128 * c
                if (c, n) not in proj_state:
                    proj_state[(c, n)] = aux.tile([128, TB], f32, tag="aux",
                                                  name="pjps")
                ps = proj_state[(c, n)][0:mc, :]
                for j in range(4 * uh, 4 * uh + 4):
                    nc.tensor.matmul(
                        ps,
                        lhsT=pins[j][:].rearrange("p a t -> p (a t)"),
                        rhs=wproj_sb[:, j, 512 * n:512 * (n + 1)],
                        start=(j == 0),
                        stop=(j == NCORES - 1) and not with_bproj)
                if uh == 0:
                    return
                if with_bproj:
                    nc.tensor.matmul(
                        ps, lhsT=ones_col[:, 0:mc],
                        rhs=bproj_sb[:, 512 * n:512 * (n + 1)],
                        start=False, stop=True)
                del proj_state[(c, n)]
                if (c, "osb") not in proj_state:
                    proj_state[(c, "osb")] = out_pool.tile(
                        [128, D], f32, tag="osb", name="osb")
                osb = proj_state[(c, "osb")]
                nc.vector.tensor_copy(out=osb[0:mc, 512 * n:512 * (n + 1)],
                                      in_=ps)
                if n == 1:
                    del proj_state[(c, "osb")]
                    del pins_by_c[c]
                    nc.sync.dma_start(out=out[r0:r0 + mc, :],
                                      in_=osb[0:mc, :])

            def proj_all(c):
                proj_half(c, 0, 0)
                proj_half(c, 0, 1)
                proj_half(c, 1, 0)
                proj_half(c, 1, 1)

            # ---- emission order ----
            # u[tb] = [dma, qk0a, qk0b, qk1a, qk1b, v0, v1, v2, v3]
            u = {tb: qkv_tblock_units(tb) for tb in range(NTB)}
            prime_cc()
            for f in u[0][:5]:    # tb0 dma+qk only; v deferred so the
                f()               # first S^T isn't queued behind it
            fill = {}

            def put(g, *fns):
                fill.setdefault(g, []).extend(fns)

            # b0 phase: just-in-time qk for tb1-3 (kt chunk 4j needed by
            # chunk 4j), V units placed PVLAG chunks before their PV
            put(0, u[0][5])
            put(1, u[1][0], u[1][1])
            put(2, u[1][2], u[1][3])
            put(3, u[1][4], u[0][6])
            put(4, u[0][7], u[0][8])
            put(5, u[2][0], u[1][5])
            put(6, u[2][1], u[2][2])
            put(7, u[2][3], u[2][4])
            put(8, u[1][6], u[1][7])
            put(9, u[3][0], u[1][8])
            put(10, u[3][1], u[3][2])
            put(11, u[3][3], u[3][4])
            put(12, u[2][5], u[2][6])
            put(13, u[2][7], u[2][8])
            put(14, u[3][5], u[3][6])
            put(15, u[3][7], u[3][8])
            # b1 QKV: no urgency, one unit per chunk
            for i, tb in enumerate((4, 5, 6, 7)):
                g0 = 18 + 10 * i
                for k in range(9):
                    put(g0 + k, u[tb][k])
            put(40, emit_wproj_loads)
            # collective c triggers at chunk 32c+37 (normalize of block
            # 2c+1) but cannot start before the CC stream frees; measured
            # lands: C0 ~chunk 72, C1 ~chunk 100.  pins/proj positions sit
            # safely after so they never head-of-line-block the strict-FIFO
            # PE queue.  proj halves are spread one per chunk so the 2-deep
            # exp pipeline never drains.
            put(66, lambda: pins_load(0))
            for i in range(4):
                put(71 + i, (lambda nn, uu: lambda: proj_half(0, nn, uu))(
                    i // 2, i % 2))
            put(101, lambda: pins_load(1))
            for i in range(4):
                put(107 + i, (lambda nn, uu: lambda: proj_half(1, nn, uu))(
                    i // 2, i % 2))
            put(130, lambda: pins_load(2))

            NG = len(CHUNKS)
            for g in range(NG + PVLAG):
                if g in fill:
                    for fn in fill[g]:
                        fn()
                if g < NG:
                    emit_st(g)
                if g >= PVLAG:
                    emit_pv(g - PVLAG)

            # tail: proj for collectives 2 (pair) and 3 (block 6) fills the
            # last collective's flight window; dummy matmuls keep the PE's
            # HAM clock warm until block 7's pins arrive
            def dummy_mms(n):
                ps = aux.tile([128, TB], f32, tag="aux", name="warm")
                for i in range(n):
                    nc.tensor.matmul(
                        ps[:],
                        lhsT=wproj_sb[:, 0, 0:128],
                        rhs=wproj_sb[:, 1, 0:512],
                        start=True, stop=True)

            proj_all(2)
            dummy_mms(12)
            pins_load(3)
            dummy_mms(12)
            proj_all(3)

    nc.finalize()
    return nc


def _prep_inputs(x, attention_mask, w_attn, b_attn, w_proj, b_proj):
    x = np.asarray(x, np.float32)
    xt = np.ascontiguousarray(x.reshape(BT, D).T).astype(BF16)
    w_attn = np.asarray(w_attn, np.float32)
    b_attn = np.asarray(b_attn, np.float32)
    wp = np.ascontiguousarray(np.asarray(w_proj, np.float32)).astype(BF16)
    scale = 1.0 / np.sqrt(HD)
    am = np.asarray(attention_mask, np.float32)
    with_mask = bool(np.any(am))
    with_battn = bool(np.any(b_attn))
    with_bproj = bool(np.any(np.asarray(b_proj)))
    mask2 = None
    if with_mask:
        m2 = (2.0 * am).reshape(B, T // 128, 128)
        mask2 = np.ascontiguousarray(m2.transpose(2, 0, 1).reshape(128, -1))
    in_maps = []
    for c in range(NCORES):
        h0, h1 = HPC * c, HPC * c + 1
        qkcols = []
        vcols = []
        for h in (h0, h1):
            qkcols.append(w_attn[:, HD * h:HD * (h + 1)] * scale)      # q
            qkcols.append(w_attn[:, D + HD * h:D + HD * (h + 1)])      # k
            vcols.append(w_attn[:, 2 * D + HD * h:2 * D + HD * (h + 1)])
        wqk = np.ascontiguousarray(np.concatenate(qkcols, axis=1)).astype(BF16)
        wv = np.ascontiguousarray(np.concatenate(vcols, axis=1)).astype(BF16)
        m = {"xt": xt, "w_qk": wqk, "w_v": wv, "w_proj": wp}
        if with_mask:
            m["mask2"] = mask2
        if with_battn:
            bqk = []
            bv = []
            for h in (h0, h1):
                bqk.append(b_attn[HD * h:HD * (h + 1)] * scale)
                bqk.append(b_attn[D + HD * h:D + HD * (h + 1)])
                bv.append(b_attn[2 * D + HD * h:2 * D + HD * (h + 1)])
            m["b_qk"] = np.ascontiguousarray(
                np.concatenate(bqk)[None, :].astype(BF16))
            m["b_v"] = np.ascontiguousarray(
                np.concatenate(bv)[None, :].astype(BF16))
        if with_bproj:
            m["b_proj"] = np.ascontiguousarray(
                np.asarray(b_proj, np.float32)[None, :].astype(BF16))
        in_maps.append(m)
    return in_maps, (with_mask, with_battn, with_bproj)


def _run(inputs, trace=False, tmpdir=None):
    from concourse.bass_utils import run_bass_kernel_spmd

    in_maps, key = _prep_inputs(**inputs)
    if key not in _CACHE:
        _CACHE[key] = _build(*key)
    nc = _CACHE[key]
    try:
        res = run_bass_kernel_spmd(nc, in_maps, core_ids=list(range(NCORES)),
                                   trace=trace, tmpdir=tmpdir)
    except Exception as e:
        if "unrecoverable" not in str(e) and "UNAVAILABLE" not in str(e):
            raise
        import ctypes
        lib = ctypes.CDLL("/opt/axon/libaxon_pjrt.so")
        if hasattr(lib, "axon_reset"):
            lib.axon_reset.restype = ctypes.c_int64
            lib.axon_reset()
        res = run_bass_kernel_spmd(nc, in_maps, core_ids=list(range(NCORES)),
                                   trace=trace, tmpdir=tmpdir)
    # local out rows: 8 chunks of 64; chunk k = 4b + qb holds global rows
    # [2048b + 512qb + 64c, +64) for core c
    y = np.empty((B, T, D), np.float32)
    for c in range(NCORES):
        o = res.results[c]["out"]
        for k in range(NBLK):
            b, qb = k // NQB, k % NQB
            r0 = QB * qb + 64 * c
            y[b, r0:r0 + 64] = o[64 * k:64 * k + 64]
    return y, res


def kernel(**inputs) -> np.ndarray:
    y, _ = _run(inputs, trace=False)
    return y


# revision 29
# speedup vs baseline: 1.2455x; 1.2455x over previous
"""Distributed multi-head attention kernel for one TRN2 chip (8 NeuronCores).

Problem: B=2, T=2048, D=1024, H=16 heads (hd=64).
  qkv = x @ w_attn + b_attn ; per-head softmax((q k^T)/sqrt(hd) + 2*mask) v
  out = attn @ w_proj + b_proj

Sharding: tensor-parallel over heads. Core c owns heads {2c, 2c+1}.
  - Q/K projections in transposed layout (QT/KT: [hd, T]); head h lives at
    partitions [64h, 64h+64) of qt2/kt2.
  - S^T[kv, q] = K_h^T Q_h as K=64 row-tiled matmul pairs: the two heads
    occupy PE row-halves (tile_position (0,0)/(64,0)) and stream
    concurrently - 2x over the zero-padded K=128 formulation.
  - V is computed directly in natural [t, hd] layout (lhsT = x chunk,
    rhs = w_v), eliminating PE transposes.
  - Softmax denominator comes from a ones-column appended to V in the PV
    matmul (O_ext = [V|1]^T @ P^T); PV is stream-bound and runs at the
    N-cycle roofline already.
  - Exp runs on ScalarE only (it is the pacing engine); all PSUM->SBUF
    copies are on VectorE.
  - Output rows are STRIPED across cores in 64-row chunks (global 64-row
    chunk R -> core R%8), so every (b, qb) q-block completes a full
    AllToAll payload for all 8 destination cores.  8 mini-AllToAlls
    ([8,128,64] bf16 each) fire as soon as each q-block normalizes; all
    but the last are fully hidden under the chunk stream, killing the
    ~50us serial tail of the 2-collective design.
  - The final projection consumes pin pairs (two mini-A2A outputs = 128
    rows, M=128 lhsT) and is scheduled into PE slack: pairs 0/1 in the
    ScalarE-bound late stream, pair 2 inside the last collective's
    flight window, pair 3 right after it lands.
  - All matmul operands bf16; accumulation + softmax statistics fp32.
  - PSUM: 2x[128,1024] S^T/exp ping-pong (4 banks) + 2x[128,512] aux
    (QKV/proj/warmup, 2 banks) + 2x[65,512] PV accumulators (2 banks).
"""

import sys

sys.path.insert(0, "/opt/trn_rl_repo")

import numpy as np

B, T, D = 2, 2048, 1024
H = 16
HD = D // H
NCORES = 8
HPC = H // NCORES          # heads per core = 2
BT = B * T                 # 4096 global rows
ROWS_PER_CORE = BT // NCORES   # 512
TB = 512                   # t-block width for QKV projection
NTB = BT // TB             # 8
NKD = D // 128             # 8 contraction chunks over D
QB = 512                   # q-block width in attention
NQB = T // QB              # 4 per (batch, head)
NKV = T // 128             # 16 kv chunks per batch
PVLAG = 6                  # PV trails exp by this many kv chunks
NBLK = B * NQB             # 8 q-blocks == 8 mini collectives

_CACHE = {}
import ml_dtypes
BF16 = ml_dtypes.bfloat16


def _build(with_mask: bool, with_battn: bool, with_bproj: bool):
    import concourse.bass as bass
    import concourse.tile as tile
    from concourse import bacc, mybir

    f32 = mybir.dt.float32
    bf16 = mybir.dt.bfloat16
    Exp = mybir.ActivationFunctionType.Exp

    nc = bacc.Bacc("TRN2", target_bir_lowering=False, debug=False,
                   num_devices=NCORES)
    rg = [list(range(NCORES))]

    xt = nc.dram_tensor("xt", [D, BT], bf16, kind="ExternalInput")
    w_qk = nc.dram_tensor("w_qk", [D, 256], bf16, kind="ExternalInput")
    w_v = nc.dram_tensor("w_v", [D, 128], bf16, kind="ExternalInput")
    w_proj = nc.dram_tensor("w_proj", [D, D], bf16, kind="ExternalInput")
    if with_mask:
        mask2 = nc.dram_tensor("mask2", [128, B * NKV], f32,
                               kind="ExternalInput")
    if with_battn:
        b_qk = nc.dram_tensor("b_qk", [1, 256], bf16, kind="ExternalInput")
        b_v = nc.dram_tensor("b_v", [1, 128], bf16, kind="ExternalInput")
    if with_bproj:
        b_proj = nc.dram_tensor("b_proj", [1, D], bf16, kind="ExternalInput")
    out = nc.dram_tensor("out", [ROWS_PER_CORE, D], f32, kind="ExternalOutput")

    with tile.TileContext(nc, num_cores=NCORES) as tc:
        from contextlib import ExitStack
        with ExitStack() as ctx:
            const = ctx.enter_context(tc.tile_pool(name="const", bufs=1))
            xt_pool = ctx.enter_context(tc.tile_pool(name="xtp", bufs=4))
            qk_pool = ctx.enter_context(tc.tile_pool(name="qkp", bufs=1))
            pt_pool = ctx.enter_context(tc.tile_pool(name="ptp", bufs=9))
            lbc_pool = ctx.enter_context(tc.tile_pool(name="lbc", bufs=2))
            pin_pool = ctx.enter_context(tc.tile_pool(name="pin", bufs=4))
            out_pool = ctx.enter_context(tc.tile_pool(name="outp", bufs=2))
            # PSUM: hot = S^T/exp ping-pong, 2 slots x [128,1024]f32
            # (2 banks each); aux = QKV/proj/warmup accumulators, 2 slots
            # x [128,512]f32 (1 bank); acc = PV accumulators 2 x [65,512].
            hot = ctx.enter_context(tc.tile_pool(name="hot", bufs=2,
                                                 space="PSUM"))
            aux = ctx.enter_context(tc.tile_pool(name="aux", bufs=2,
                                                 space="PSUM"))
            acc_pool = ctx.enter_context(tc.tile_pool(name="accp", bufs=2,
                                                      space="PSUM"))
            dram = ctx.enter_context(tc.tile_pool(name="dram", bufs=4,
                                                  space="DRAM"))

            # ---- ACT exp-table preload (runs at t~0 on the scalar queue) --
            dmy = const.tile([1, 8], bf16)
            nc.vector.memset(dmy[:], 0.0)
            dmy2 = const.tile([1, 8], f32)
            nc.scalar.activation(out=dmy2[:], in_=dmy[:], func=Exp)

            # ---- PE warmup: dependency-free matmuls run during the input
            # DMA wait, flipping the HAM clock gate to full rate before the
            # first real QKV matmuls issue.
            wrm = const.tile([128, TB], bf16)
            nc.vector.memset(wrm[:], 0.0)
            wps = [aux.tile([128, TB], f32, tag="aux", name="wps")
                   for _ in range(2)]
            for i in range(10):
                nc.tensor.matmul(
                    wps[i % 2][:],
                    lhsT=wrm[:, 0:128], rhs=wrm[:],
                    start=True, stop=True)

            # ---- constants ----
            # w_qk packed per D-chunk: [128, NKD, 256]; group g cols
            # [128g, 128g+128): g0=[q_h0/8|k_h0] g1=[q_h1/8|k_h1]
            wqk_sb = const.tile([128, NKD, 256], bf16)
            for half in range(2):
                nc.sync.dma_start(
                    out=wqk_sb[:, 4 * half:4 * (half + 1), :],
                    in_=w_qk[:].rearrange("(a p) c -> p a c", p=128)[
                        :, 4 * half:4 * (half + 1), :])
            # w_v packed per D-chunk: [128, NKD, 128]; cols [v_h0|v_h1]
            wv_sb = const.tile([128, NKD, 128], bf16)
            nc.scalar.dma_start(
                out=wv_sb[:],
                in_=w_v[:].rearrange("(a p) c -> p a c", p=128))
            wproj_sb = const.tile([128, NKD, D], bf16)

            def emit_wproj_loads():
                for half in range(2):
                    nc.sync.dma_start(
                        out=wproj_sb[:, 4 * half:4 * (half + 1), :],
                        in_=w_proj[:].rearrange("(a p) c -> p a c", p=128)[
                            :, 4 * half:4 * (half + 1), :])

            if with_mask:
                # mask (already doubled on host): [128, B, NKV]
                mask_sb = const.tile([128, B, NKV], f32)
                nc.sync.dma_start(out=mask_sb[:],
                                  in_=mask2[:].rearrange("p (b j) -> p b j",
                                                         b=B))
            if with_battn:
                bqk_sb = const.tile([1, 256], bf16)
                nc.sync.dma_start(out=bqk_sb[:], in_=b_qk[:])
                bv_sb = const.tile([1, 128], bf16)
                nc.sync.dma_start(out=bv_sb[:], in_=b_v[:])
                ones_row = const.tile([1, TB], bf16)
                nc.vector.memset(ones_row[:], 1.0)
            if with_bproj:
                bproj_sb = const.tile([1, D], bf16)
                nc.sync.dma_start(out=bproj_sb[:], in_=b_proj[:])
                ones_col = const.tile([1, 128], bf16)
                nc.vector.memset(ones_col[:], 1.0)

            ones64 = const.tile([1, 64], bf16)
            nc.vector.memset(ones64[:], 1.0)

            # persistent activations. Head h at partitions [64h, 64h+64).
            qt2 = qk_pool.tile([128, BT], bf16, tag="qt2", name="qt2")
            kt2 = qk_pool.tile([128, BT], bf16, tag="kt2", name="kt2")
            ot = qk_pool.tile([128, BT], bf16, tag="ot", name="ot")
            # V natural layout + ones column: subtile s = 32b + 2j + h is
            # [128 kv, 65] = [V_chunk | 1].
            vnat = qk_pool.tile([128, 2 * NKV * HPC, 65], bf16, tag="vnat",
                                name="vnat")
            nc.vector.memset(vnat[:, :, 64:65], 1.0)

            # ---- QKV projection ----
            # q/k: transposed outputs via stationary weights.
            # v: natural output via stationary x chunks.
            def qkv_tblock_units(tb):
                xt_t = xt_pool.tile([128, NKD, TB], bf16, tag="xt", name="xt")

                def dma_unit():
                    xsrc = xt[:].rearrange("(a p) t -> p a t", p=128)
                    for half in range(2):
                        nc.sync.dma_start(
                            out=xt_t[:, 4 * half:4 * (half + 1), :],
                            in_=xsrc[:, 4 * half:4 * (half + 1),
                                     TB * tb:TB * (tb + 1)])

                # finer-grained halves so PE-queue fillers never stall
                # the scalar engine for more than ~1us
                qkps = {}

                def qk_half(g, half):
                    if g not in qkps:
                        qkps[g] = aux.tile([128, TB], f32, tag="aux",
                                           name="qkps")
                    ps = qkps[g]
                    for d in range(4 * half, 4 * half + 4):
                        nc.tensor.matmul(
                            ps[:],
                            lhsT=wqk_sb[:, d, 128 * g:128 * (g + 1)],
                            rhs=xt_t[:, d, :],
                            start=(d == 0),
                            stop=(d == NKD - 1) and not with_battn)
                    if half == 0:
                        return
                    if with_battn:
                        nc.tensor.matmul(
                            ps[:],
                            lhsT=bqk_sb[:, 128 * g:128 * (g + 1)],
                            rhs=ones_row[:],
                            start=False, stop=True)
                    cs = TB * tb
                    nc.vector.tensor_copy(
                        out=qt2[64 * g:64 * (g + 1), cs:cs + TB],
                        in_=ps[0:64, :])
                    nc.vector.tensor_copy(
                        out=kt2[64 * g:64 * (g + 1), cs:cs + TB],
                        in_=ps[64:128, :])
                    del qkps[g]

                def v_chunk(c4):
                    ps = aux.tile([128, TB], f32, tag="aux", name="vps")
                    for d in range(NKD):
                        nc.tensor.matmul(
                            ps[:, 0:128],
                            lhsT=xt_t[:, d, 128 * c4:128 * (c4 + 1)],
                            rhs=wv_sb[:, d, :],
                            start=(d == 0),
                            stop=(d == NKD - 1) and not with_battn)
                    if with_battn:
                        nc.tensor.matmul(
                            ps[:, 0:128],
                            lhsT=ones_row[:, 0:128],
                            rhs=bv_sb[:],
                            start=False, stop=True)
                    gj = 4 * tb + c4          # global 128-row chunk id
                    b_, j = gj // NKV, gj % NKV
                    for h in range(HPC):
                        s = 32 * b_ + 2 * j + h
                        nc.vector.tensor_copy(
                            out=vnat[:, s, 0:64],
                            in_=ps[:, 64 * h:64 * (h + 1)])

                return [dma_unit,
                        lambda: qk_half(0, 0), lambda: qk_half(0, 1),
                        lambda: qk_half(1, 0), lambda: qk_half(1, 1),
                        lambda: v_chunk(0), lambda: v_chunk(1),
                        lambda: v_chunk(2), lambda: v_chunk(3)]

            # ---- attention: one global software-pipelined chunk stream ----
            # chunk g = (b, qb, j); S^T+exp at position g, PV at g+PVLAG.
            CHUNKS = [(b, qb, j) for b in range(B) for qb in range(NQB)
                      for j in range(NKV)]
            pts = {}
            ps_o_by_q = {}

            def emit_st(g):
                b, qb, j = CHUNKS[g]
                c0 = 2048 * b + QB * qb
                k0 = 2048 * b + 128 * j
                # both heads' S^T chunk in one 2-bank slot; the two K=64
                # row-tiled matmuls hit disjoint PE row groups and
                # disjoint banks -> concurrent
                st = hot.tile([128, 2 * QB], f32, tag="hot", name="st")
                for h in range(HPC):
                    nc.tensor.matmul(
                        st[:, QB * h:QB * (h + 1)],
                        lhsT=kt2[64 * h:64 * (h + 1), k0:k0 + 128],
                        rhs=qt2[64 * h:64 * (h + 1), c0:c0 + QB],
                        start=True, stop=True)
                pt = pt_pool.tile([128, 2 * QB], bf16, tag="pt", name="pt")
                if with_mask:
                    nc.scalar.activation(out=pt[:], in_=st[:], func=Exp,
                                         bias=mask_sb[:, b, j:j + 1],
                                         scale=1.0)
                else:
                    nc.scalar.activation(out=pt[:], in_=st[:], func=Exp)
                pts[g] = pt

            def emit_pv(g):
                b, qb, j = CHUNKS[g]
                if j == 0:
                    ps_o_by_q[(b, qb)] = [
                        acc_pool.tile([65, QB], f32, tag="acc", name="acc")
                        for _ in range(HPC)]
                ps_o = ps_o_by_q[(b, qb)]
                pt = pts.pop(g)
                for h in range(HPC):
                    s = 32 * b + 2 * j + h
                    nc.tensor.matmul(
                        ps_o[h][:],
                        lhsT=vnat[:, s, :],
                        rhs=pt[:, QB * h:QB * (h + 1)],
                        start=(j == 0), stop=(j == NKV - 1),
                        skip_group_check=True)
                if j == NKV - 1:
                    normalize(b, qb)

            def normalize(b, qb):
                # drain ps_o to SBUF with merged tiles: both heads' O^T
                # into one [128, QB] osum (2 copies, freeing both PSUM
                # banks fast), both denominators into one [2, QB] lsb;
                # then ONE recip, ONE DRAM-broadcast roundtrip (3 DMAs)
                # and ONE [128, QB] mul produce normalized ot.
                c0 = 2048 * b + QB * qb
                ps_o = ps_o_by_q.pop((b, qb))
                osum = lbc_pool.tile([128, QB], f32, tag="osum", name="osum")
                ldram = dram.tile([2, QB], f32, tag="ld", name="ld")
                for h in range(HPC):
                    nc.vector.tensor_copy(out=osum[64 * h:64 * (h + 1), :],
                                          in_=ps_o[h][0:64, :])
                    lsb = lbc_pool.tile([1, QB], f32, tag="lsb", name="lsb")
                    nc.vector.tensor_copy(out=lsb[:], in_=ps_o[h][64:65, :])
                    lrec = lbc_pool.tile([1, QB], f32, tag="lrec",
                                         name="lrec")
                    nc.vector.reciprocal_approx_fast(out=lrec[:],
                                                     in_=lsb[:])
                    nc.sync.dma_start(out=ldram[h:h + 1, :], in_=lrec[:])
                lbc = lbc_pool.tile([128, QB], f32, tag="lbc", name="lbc")
                for h in range(HPC):
                    nc.sync.dma_start(
                        out=lbc[64 * h:64 * (h + 1), :],
                        in_=ldram[h:h + 1, :].to_broadcast([64, QB]))
                nc.vector.tensor_mul(out=ot[:, c0:c0 + QB], in0=osum[:],
                                     in1=lbc[:])
                a2a_block(b, qb)

            # ---- striped mini-AllToAll (head-space -> row-space) ----
            # q-block (b, qb) == block k = 4b + qb.  Dest core j receives
            # ot cols [2048b + 512qb + 64j, +64) from every core and owns
            # local out rows [64k, 64k+64).  Blocks are shipped in PAIRS
            # (one collective per 128 output rows): slot j = [128, 2, 64].
            from concourse import mybir as _mb

            def prime_cc():
                # tiny dummy AllToAll (garbage data, output unused) so the
                # CC stream's one-time barrier + init (~30-45us) runs
                # during the PE-bound QKV phase
                pin = dram.tile([NCORES, 1, 64], bf16, tag="a2ain",
                                name="prime_in")
                pout = dram.tile([NCORES, 1, 64], bf16, tag="a2aout",
                                 name="prime_out")
                nc.gpsimd.collective_compute(
                    "AllToAll", _mb.AluOpType.bypass, replica_groups=rg,
                    ins=[pin.opt()], outs=[pout.opt()])

            a_ins = {}
            a2a_outs = {}

            def a2a_block(b, qb):
                # blocks ship in pairs: collective c covers blocks (2c,
                # 2c+1) == output rows [128c, 128c+128).  ONE consolidated
                # DMA per block: the j-scatter runs on the DRAM side where
                # any stride order is legal.
                k = NQB * b + qb
                c, idx = k // 2, k % 2
                if idx == 0:
                    a_ins[c] = dram.tile([NCORES, 128, 2, 64], bf16,
                                         tag="a2ain", name="a2ain")
                a_in = a_ins[c]
                base = 2048 * b + QB * qb
                nc.sync.dma_start(
                    out=a_in[:, :, idx, :].rearrange("j p t -> p j t"),
                    in_=ot[:, base:base + QB].rearrange("p (j t) -> p j t",
                                                        j=NCORES))
                if idx == 0:
                    return
                del a_ins[c]
                a_out = dram.tile([NCORES, 128, 2, 64], bf16, tag="a2aout",
                                  name="a2aout")
                nc.gpsimd.collective_compute(
                    "AllToAll", _mb.AluOpType.bypass, replica_groups=rg,
                    ins=[a_in.opt()], outs=[a_out.opt()])
                a2a_outs[c] = a_out

            # pins: collective c covers out rows [128c, 128c+128); slice
            # [:, j, :, :] is the [128, 128] lhsT chunk for contraction
            # block j.
            pins_by_c = {}

            def pins_load(c):
                # one consolidated DMA on the gpsimd queue, so the
                # wait-for-collective never head-of-line-blocks the sync
                # queue's normalize DMAs or any compute engine
                a_out = a2a_outs.pop(c)
                pinbig = pin_pool.tile([128, NCORES, 2, 64], bf16,
                                       tag="pin", name="pin")
                nc.gpsimd.dma_start(
                    out=pinbig[:],
                    in_=a_out[:].rearrange("j p a t -> p j a t"))
                pins_by_c[c] = pinbig

            proj_state = {}

            def proj_half(c, n, uh):
                # collective c -> out rows [128c, 128c+128), cols [512n,
                # +512), contraction blocks j in [4uh, 4uh+4)
                pins = pins_by_c[c]
                mc, r0 = 128, 128 * c
                if (c, n) not in proj_state:
                    proj_state[(c, n)] = aux.tile([128, TB], f32, tag="aux",
                                                  name="pjps")
                ps = proj_state[(c, n)][0:mc, :]
                for j in range(4 * uh, 4 * uh + 4):
                    nc.tensor.matmul(
                        ps,
                        lhsT=pins[:, j, :, :].rearrange("p a t -> p (a t)"),
                        rhs=wproj_sb[:, j, 512 * n:512 * (n + 1)],
                        start=(j == 0),
                        stop=(j == NCORES - 1) and not with_bproj)
                if uh == 0:
                    return
                if with_bproj:
                    nc.tensor.matmul(
                        ps, lhsT=ones_col[:, 0:mc],
                        rhs=bproj_sb[:, 512 * n:512 * (n + 1)],
                        start=False, stop=True)
                del proj_state[(c, n)]
                if (c, "osb") not in proj_state:
                    proj_state[(c, "osb")] = out_pool.tile(
                        [128, D], f32, tag="osb", name="osb")
                osb = proj_state[(c, "osb")]
                nc.vector.tensor_copy(out=osb[0:mc, 512 * n:512 * (n + 1)],
                                      in_=ps)
                if n == 1:
                    del proj_state[(c, "osb")]
                    del pins_by_c[c]
                    nc.sync.dma_start(out=out[r0:r0 + mc, :],
                                      in_=osb[0:mc, :])

            def proj_all(c):
                proj_half(c, 0, 0)
                proj_half(c, 0, 1)
                proj_half(c, 1, 0)
                proj_half(c, 1, 1)

            # ---- emission order ----
            # u[tb] = [dma, qk0a, qk0b, qk1a, qk1b, v0, v1, v2, v3]
            u = {tb: qkv_tblock_units(tb) for tb in range(NTB)}
            prime_cc()
            for f in u[0][:5]:    # tb0 dma+qk only; v deferred so the
                f()               # first S^T isn't queued behind it
            fill = {}

            def put(g, *fns):
                fill.setdefault(g, []).extend(fns)

            # b0 phase: just-in-time qk for tb1-3 (kt chunk 4j needed by
            # chunk 4j), V units placed PVLAG chunks before their PV
            put(0, u[0][5])
            put(1, u[1][0], u[1][1])
            put(2, u[1][2], u[1][3])
            put(3, u[1][4], u[0][6])
            put(4, u[0][7], u[0][8])
            put(5, u[2][0], u[1][5])
            put(6, u[2][1], u[2][2])
            put(7, u[2][3], u[2][4])
            put(8, u[1][6], u[1][7])
            put(9, u[3][0], u[1][8])
            put(10, u[3][1], u[3][2])
            put(11, u[3][3], u[3][4])
            put(12, u[2][5], u[2][6])
            put(13, u[2][7], u[2][8])
            put(14, u[3][5], u[3][6])
            put(15, u[3][7], u[3][8])
            # b1 QKV: no urgency, one unit per chunk
            for i, tb in enumerate((4, 5, 6, 7)):
                g0 = 18 + 10 * i
                for k in range(9):
                    put(g0 + k, u[tb][k])
            put(40, emit_wproj_loads)
            # collective c triggers at chunk 32c+37 (normalize of block
            # 2c+1) but cannot start before the CC stream frees; measured
            # lands: C0 ~chunk 72, C1 ~chunk 100.  pins/proj positions sit
            # safely after so they never head-of-line-block the strict-FIFO
            # PE queue.  proj halves are spread one per chunk so the 2-deep
            # exp pipeline never drains.
            put(66, lambda: pins_load(0))
            for i in range(4):
                put(71 + i, (lambda nn, uu: lambda: proj_half(0, nn, uu))(
                    i // 2, i % 2))
            put(101, lambda: pins_load(1))
            for i in range(4):
                put(107 + i, (lambda nn, uu: lambda: proj_half(1, nn, uu))(
                    i // 2, i % 2))
            put(130, lambda: pins_load(2))

            NG = len(CHUNKS)
            for g in range(NG + PVLAG):
                if g in fill:
                    for fn in fill[g]:
                        fn()
                if g < NG:
                    emit_st(g)
                if g >= PVLAG:
                    emit_pv(g - PVLAG)

            # tail: proj for collectives 2 (pair) and 3 (block 6) fills the
            # last collective's flight window; dummy matmuls keep the PE's
            # HAM clock warm until block 7's pins arrive
            def dummy_mms(n):
                ps = aux.tile([128, TB], f32, tag="aux", name="warm")
                for i in range(n):
                    nc.tensor.matmul(
                        ps[:],
                        lhsT=wproj_sb[:, 0, 0:128],
                        rhs=wproj_sb[:, 1, 0:512],
                        start=True, stop=True)

            proj_all(2)
            dummy_mms(12)
            pins_load(3)
            dummy_mms(12)
            proj_all(3)

    nc.finalize()
    return nc


def _prep_inputs(x, attention_mask, w_attn, b_attn, w_proj, b_proj):
    x = np.asarray(x, np.float32)
    xt = np.ascontiguousarray(x.reshape(BT, D).T).astype(BF16)
    w_attn = np.asarray(w_attn, np.float32)
    b_attn = np.asarray(b_attn, np.float32)
    wp = np.ascontiguousarray(np.asarray(w_proj, np.float32)).astype(BF16)
    scale = 1.0 / np.sqrt(HD)
    am = np.asarray(attention_mask, np.float32)
    with_mask = bool(np.any(am))
    with_battn = bool(np.any(b_attn))
    with_bproj = bool(np.any(np.asarray(b_proj)))
    mask2 = None
    if with_mask:
        m2 = (2.0 * am).reshape(B, T // 128, 128)
        mask2 = np.ascontiguousarray(m2.transpose(2, 0, 1).reshape(128, -1))
    in_maps = []
    for c in range(NCORES):
        h0, h1 = HPC * c, HPC * c + 1
        qkcols = []
        vcols = []
        for h in (h0, h1):
            qkcols.append(w_attn[:, HD * h:HD * (h + 1)] * scale)      # q
            qkcols.append(w_attn[:, D + HD * h:D + HD * (h + 1)])      # k
            vcols.append(w_attn[:, 2 * D + HD * h:2 * D + HD * (h + 1)])
        wqk = np.ascontiguousarray(np.concatenate(qkcols, axis=1)).astype(BF16)
        wv = np.ascontiguousarray(np.concatenate(vcols, axis=1)).astype(BF16)
        m = {"xt": xt, "w_qk": wqk, "w_v": wv, "w_proj": wp}
        if with_mask:
            m["mask2"] = mask2
        if with_battn:
            bqk = []
            bv = []
            for h in (h0, h1):
                bqk.append(b_attn[HD * h:HD * (h + 1)] * scale)
                bqk.append(b_attn[D + HD * h:D + HD * (h + 1)])
                bv.append(b_attn[2 * D + HD * h:2 * D + HD * (h + 1)])
            m["b_qk"] = np.ascontiguousarray(
                np.concatenate(bqk)[None, :].astype(BF16))
            m["b_v"] = np.ascontiguousarray(
                np.concatenate(bv)[None, :].astype(BF16))
        if with_bproj:
            m["b_proj"] = np.ascontiguousarray(
                np.asarray(b_proj, np.float32)[None, :].astype(BF16))
        in_maps.append(m)
    return in_maps, (with_mask, with_battn, with_bproj)


def _run(inputs, trace=False, tmpdir=None):
    from concourse.bass_utils import run_bass_kernel_spmd

    in_maps, key = _prep_inputs(**inputs)
    if key not in _CACHE:
        _CACHE[key] = _build(*key)
    nc = _CACHE[key]
    try:
        res = run_bass_kernel_spmd(nc, in_maps, core_ids=list(range(NCORES)),
                                   trace=trace, tmpdir=tmpdir)
    except Exception as e:
        if "unrecoverable" not in str(e) and "UNAVAILABLE" not in str(e):
            raise
        import ctypes
        lib = ctypes.CDLL("/opt/axon/libaxon_pjrt.so")
        if hasattr(lib, "axon_reset"):
            lib.axon_reset.restype = ctypes.c_int64
            lib.axon_reset()
        res = run_bass_kernel_spmd(nc, in_maps, core_ids=list(range(NCORES)),
                                   trace=trace, tmpdir=tmpdir)
    # local out rows: 8 chunks of 64; chunk k = 4b + qb holds global rows
    # [2048b + 512qb + 64c, +64) for core c
    y = np.empty((B, T, D), np.float32)
    for c in range(NCORES):
        o = res.results[c]["out"]
        for k in range(NBLK):
            b, qb = k // NQB, k % NQB
            r0 = QB * qb + 64 * c
            y[b, r0:r0 + 64] = o[64 * k:64 * k + 64]
    return y, res


def kernel(**inputs) -> np.ndarray:
    y, _ = _run(inputs, trace=False)
    return y


# revision 36
# speedup vs baseline: 1.3675x; 1.0980x over previous
"""Distributed multi-head attention kernel for one TRN2 chip (8 NeuronCores).

Problem: B=2, T=2048, D=1024, H=16 heads (hd=64).
  qkv = x @ w_attn + b_attn ; per-head softmax((q k^T)/sqrt(hd) + 2*mask) v
  out = attn @ w_proj + b_proj

Sharding: tensor-parallel over heads. Core c owns heads {2c, 2c+1}.
  - Q/K projections in transposed layout (QT/KT: [hd, T]); head h lives at
    partitions [64h, 64h+64) of qt2/kt2.
  - S^T[kv, q] = K_h^T Q_h as K=64 row-tiled matmul pairs: the two heads
    occupy PE row-halves (tile_position (0,0)/(64,0)) and stream
    concurrently - 2x over the zero-padded K=128 formulation.
  - V is computed directly in natural [t, hd] layout (lhsT = x chunk,
    rhs = w_v), eliminating PE transposes.
  - Softmax denominator comes from a ones-column appended to V in the PV
    matmul (O_ext = [V|1]^T @ P^T); PV is stream-bound and runs at the
    N-cycle roofline already.
  - Exp runs on ScalarE only (it is the pacing engine); all PSUM->SBUF
    copies are on VectorE.
  - Output rows are STRIPED across cores in 64-row chunks (global 64-row
    chunk R -> core R%8), so every (b, qb) q-block completes a full
    AllToAll payload for all 8 destination cores.  8 mini-AllToAlls
    ([8,128,64] bf16 each) fire as soon as each q-block normalizes; all
    but the last are fully hidden under the chunk stream, killing the
    ~50us serial tail of the 2-collective design.
  - The final projection consumes pin pairs (two mini-A2A outputs = 128
    rows, M=128 lhsT) and is scheduled into PE slack: pairs 0/1 in the
    ScalarE-bound late stream, pair 2 inside the last collective's
    flight window, pair 3 right after it lands.
  - All matmul operands bf16; accumulation + softmax statistics fp32.
  - PSUM: 2x[128,1024] S^T/exp ping-pong (4 banks) + 2x[128,512] aux
    (QKV/proj/warmup, 2 banks) + 2x[65,512] PV accumulators (2 banks).
"""

import sys

sys.path.insert(0, "/opt/trn_rl_repo")

import numpy as np

B, T, D = 2, 2048, 1024
H = 16
HD = D // H
NCORES = 8
HPC = H // NCORES          # heads per core = 2
BT = B * T                 # 4096 global rows
ROWS_PER_CORE = BT // NCORES   # 512
TB = 512                   # t-block width for QKV projection
NTB = BT // TB             # 8
NKD = D // 128             # 8 contraction chunks over D
QB = 512                   # q-block width in attention
NQB = T // QB              # 4 per (batch, head)
NKV = T // 128             # 16 kv chunks per batch
PVLAG = 6                  # PV trails exp by this many kv chunks
NBLK = B * NQB             # 8 q-blocks == 8 mini collectives

_CACHE = {}
import ml_dtypes
BF16 = ml_dtypes.bfloat16


def _build(with_mask: bool, with_battn: bool, with_bproj: bool):
    import concourse.bass as bass
    import concourse.tile as tile
    from concourse import bacc, mybir

    f32 = mybir.dt.float32
    bf16 = mybir.dt.bfloat16
    Exp = mybir.ActivationFunctionType.Exp

    nc = bacc.Bacc("TRN2", target_bir_lowering=False, debug=False,
                   num_devices=NCORES)
    rg = [list(range(NCORES))]

    xt = nc.dram_tensor("xt", [D, BT], bf16, kind="ExternalInput")
    w_qk = nc.dram_tensor("w_qk", [D, 256], bf16, kind="ExternalInput")
    w_v = nc.dram_tensor("w_v", [D, 128], bf16, kind="ExternalInput")
    w_proj = nc.dram_tensor("w_proj", [D, D], bf16, kind="ExternalInput")
    if with_mask:
        mask2 = nc.dram_tensor("mask2", [128, B * NKV], f32,
                               kind="ExternalInput")
    if with_battn:
        b_qk = nc.dram_tensor("b_qk", [1, 256], bf16, kind="ExternalInput")
        b_v = nc.dram_tensor("b_v", [1, 128], bf16, kind="ExternalInput")
    if with_bproj:
        b_proj = nc.dram_tensor("b_proj", [1, D], bf16, kind="ExternalInput")
    out = nc.dram_tensor("out", [ROWS_PER_CORE, D], f32, kind="ExternalOutput")

    with tile.TileContext(nc, num_cores=NCORES) as tc:
        from contextlib import ExitStack
        with ExitStack() as ctx:
            const = ctx.enter_context(tc.tile_pool(name="const", bufs=1))
            xt_pool = ctx.enter_context(tc.tile_pool(name="xtp", bufs=4))
            qk_pool = ctx.enter_context(tc.tile_pool(name="qkp", bufs=1))
            pt_pool = ctx.enter_context(tc.tile_pool(name="ptp", bufs=9))
            lbc_pool = ctx.enter_context(tc.tile_pool(name="lbc", bufs=2))
            pin_pool = ctx.enter_context(tc.tile_pool(name="pin", bufs=12))
            out_pool = ctx.enter_context(tc.tile_pool(name="outp", bufs=2))
            # PSUM: hot = S^T/exp ping-pong, 2 slots x [128,1024]f32
            # (2 banks each); aux = QKV/proj/warmup accumulators, 2 slots
            # x [128,512]f32 (1 bank); acc = PV accumulators 2 x [65,512].
            hot = ctx.enter_context(tc.tile_pool(name="hot", bufs=2,
                                                 space="PSUM"))
            aux = ctx.enter_context(tc.tile_pool(name="aux", bufs=2,
                                                 space="PSUM"))
            acc_pool = ctx.enter_context(tc.tile_pool(name="accp", bufs=2,
                                                      space="PSUM"))
            dram = ctx.enter_context(tc.tile_pool(name="dram", bufs=4,
                                                  space="DRAM"))

            # ---- ACT exp-table preload (runs at t~0 on the scalar queue) --
            dmy = const.tile([1, 8], bf16)
            nc.vector.memset(dmy[:], 0.0)
            dmy2 = const.tile([1, 8], f32)
            nc.scalar.activation(out=dmy2[:], in_=dmy[:], func=Exp)

            # ---- PE warmup: dependency-free matmuls run during the input
            # DMA wait, flipping the HAM clock gate to full rate before the
            # first real QKV matmuls issue.
            wrm = const.tile([128, TB], bf16)
            nc.vector.memset(wrm[:], 0.0)
            wps = [aux.tile([128, TB], f32, tag="aux", name="wps")
                   for _ in range(2)]
            for i in range(10):
                nc.tensor.matmul(
                    wps[i % 2][:],
                    lhsT=wrm[:, 0:128], rhs=wrm[:],
                    start=True, stop=True)

            # ---- constants ----
            # w_qk packed per D-chunk: [128, NKD, 256]; group g cols
            # [128g, 128g+128): g0=[q_h0/8|k_h0] g1=[q_h1/8|k_h1]
            wqk_sb = const.tile([128, NKD, 256], bf16)
            for half in range(2):
                nc.sync.dma_start(
                    out=wqk_sb[:, 4 * half:4 * (half + 1), :],
                    in_=w_qk[:].rearrange("(a p) c -> p a c", p=128)[
                        :, 4 * half:4 * (half + 1), :])
            # w_v packed per D-chunk: [128, NKD, 128]; cols [v_h0|v_h1]
            wv_sb = const.tile([128, NKD, 128], bf16)
            nc.scalar.dma_start(
                out=wv_sb[:],
                in_=w_v[:].rearrange("(a p) c -> p a c", p=128))
            wproj_sb = const.tile([128, NKD, D], bf16)

            def emit_wproj_loads():
                for half in range(2):
                    nc.sync.dma_start(
                        out=wproj_sb[:, 4 * half:4 * (half + 1), :],
                        in_=w_proj[:].rearrange("(a p) c -> p a c", p=128)[
                            :, 4 * half:4 * (half + 1), :])

            if with_mask:
                # mask (already doubled on host): [128, B, NKV]
                mask_sb = const.tile([128, B, NKV], f32)
                nc.sync.dma_start(out=mask_sb[:],
                                  in_=mask2[:].rearrange("p (b j) -> p b j",
                                                         b=B))
            if with_battn:
                bqk_sb = const.tile([1, 256], bf16)
                nc.sync.dma_start(out=bqk_sb[:], in_=b_qk[:])
                bv_sb = const.tile([1, 128], bf16)
                nc.sync.dma_start(out=bv_sb[:], in_=b_v[:])
                ones_row = const.tile([1, TB], bf16)
                nc.vector.memset(ones_row[:], 1.0)
            if with_bproj:
                bproj_sb = const.tile([1, D], bf16)
                nc.sync.dma_start(out=bproj_sb[:], in_=b_proj[:])
                ones_col = const.tile([1, 128], bf16)
                nc.vector.memset(ones_col[:], 1.0)

            ones64 = const.tile([1, 64], bf16)
            nc.vector.memset(ones64[:], 1.0)

            # persistent activations. Head h at partitions [64h, 64h+64).
            qt2 = qk_pool.tile([128, BT], bf16, tag="qt2", name="qt2")
            kt2 = qk_pool.tile([128, BT], bf16, tag="kt2", name="kt2")
            ot = qk_pool.tile([128, BT], bf16, tag="ot", name="ot")
            # V natural layout + ones column: subtile s = 32b + 2j + h is
            # [128 kv, 65] = [V_chunk | 1].
            vnat = qk_pool.tile([128, 2 * NKV * HPC, 65], bf16, tag="vnat",
                                name="vnat")
            nc.vector.memset(vnat[:, :, 64:65], 1.0)

            # ---- QKV projection ----
            # q/k: transposed outputs via stationary weights.
            # v: natural output via stationary x chunks.
            def qkv_tblock_units(tb):
                xt_t = xt_pool.tile([128, NKD, TB], bf16, tag="xt", name="xt")

                def dma_unit():
                    xsrc = xt[:].rearrange("(a p) t -> p a t", p=128)
                    for half in range(2):
                        nc.sync.dma_start(
                            out=xt_t[:, 4 * half:4 * (half + 1), :],
                            in_=xsrc[:, 4 * half:4 * (half + 1),
                                     TB * tb:TB * (tb + 1)])

                # finer-grained halves so PE-queue fillers never stall
                # the scalar engine for more than ~1us
                qkps = {}

                def qk_half(g, half):
                    if g not in qkps:
                        qkps[g] = aux.tile([128, TB], f32, tag="aux",
                                           name="qkps")
                    ps = qkps[g]
                    for d in range(4 * half, 4 * half + 4):
                        nc.tensor.matmul(
                            ps[:],
                            lhsT=wqk_sb[:, d, 128 * g:128 * (g + 1)],
                            rhs=xt_t[:, d, :],
                            start=(d == 0),
                            stop=(d == NKD - 1) and not with_battn)
                    if half == 0:
                        return
                    if with_battn:
                        nc.tensor.matmul(
                            ps[:],
                            lhsT=bqk_sb[:, 128 * g:128 * (g + 1)],
                            rhs=ones_row[:],
                            start=False, stop=True)
                    cs = TB * tb
                    nc.vector.tensor_copy(
                        out=qt2[64 * g:64 * (g + 1), cs:cs + TB],
                        in_=ps[0:64, :])
                    nc.vector.tensor_copy(
                        out=kt2[64 * g:64 * (g + 1), cs:cs + TB],
                        in_=ps[64:128, :])
                    del qkps[g]

                def v_chunk(c4):
                    ps = aux.tile([128, TB], f32, tag="aux", name="vps")
                    for d in range(NKD):
                        nc.tensor.matmul(
                            ps[:, 0:128],
                            lhsT=xt_t[:, d, 128 * c4:128 * (c4 + 1)],
                            rhs=wv_sb[:, d, :],
                            start=(d == 0),
                            stop=(d == NKD - 1) and not with_battn)
                    if with_battn:
                        nc.tensor.matmul(
                            ps[:, 0:128],
                            lhsT=ones_row[:, 0:128],
                            rhs=bv_sb[:],
                            start=False, stop=True)
                    gj = 4 * tb + c4          # global 128-row chunk id
                    b_, j = gj // NKV, gj % NKV
                    for h in range(HPC):
                        s = 32 * b_ + 2 * j + h
                        nc.vector.tensor_copy(
                            out=vnat[:, s, 0:64],
                            in_=ps[:, 64 * h:64 * (h + 1)])

                return [dma_unit,
                        lambda: qk_half(0, 0), lambda: qk_half(0, 1),
                        lambda: qk_half(1, 0), lambda: qk_half(1, 1),
                        lambda: v_chunk(0), lambda: v_chunk(1),
                        lambda: v_chunk(2), lambda: v_chunk(3)]

            # ---- attention: one global software-pipelined chunk stream ----
            # chunk g = (b, qb, j); S^T+exp at position g, PV at g+PVLAG.
            CHUNKS = [(b, qb, j) for b in range(B) for qb in range(NQB)
                      for j in range(NKV)]
            pts = {}
            ps_o_by_q = {}

            def emit_st(g):
                b, qb, j = CHUNKS[g]
                c0 = 2048 * b + QB * qb
                k0 = 2048 * b + 128 * j
                # both heads' S^T chunk in one 2-bank slot; the two K=64
                # row-tiled matmuls hit disjoint PE row groups and
                # disjoint banks -> concurrent
                st = hot.tile([128, 2 * QB], f32, tag="hot", name="st")
                for h in range(HPC):
                    nc.tensor.matmul(
                        st[:, QB * h:QB * (h + 1)],
                        lhsT=kt2[64 * h:64 * (h + 1), k0:k0 + 128],
                        rhs=qt2[64 * h:64 * (h + 1), c0:c0 + QB],
                        start=True, stop=True)
                pt = pt_pool.tile([128, 2 * QB], bf16, tag="pt", name="pt")
                if with_mask:
                    nc.scalar.activation(out=pt[:], in_=st[:], func=Exp,
                                         bias=mask_sb[:, b, j:j + 1],
                                         scale=1.0)
                else:
                    nc.scalar.activation(out=pt[:], in_=st[:], func=Exp)
                pts[g] = pt

            def emit_pv(g):
                b, qb, j = CHUNKS[g]
                if j == 0:
                    ps_o_by_q[(b, qb)] = [
                        acc_pool.tile([65, QB], f32, tag="acc", name="acc")
                        for _ in range(HPC)]
                ps_o = ps_o_by_q[(b, qb)]
                pt = pts.pop(g)
                for h in range(HPC):
                    s = 32 * b + 2 * j + h
                    nc.tensor.matmul(
                        ps_o[h][:],
                        lhsT=vnat[:, s, :],
                        rhs=pt[:, QB * h:QB * (h + 1)],
                        start=(j == 0), stop=(j == NKV - 1),
                        skip_group_check=True)
                if j == NKV - 1:
                    normalize(b, qb)

            def normalize(b, qb):
                # Blocks 0-3 (collectives 0/1, mid-stream): source-side
                # normalization -- merged osum drain, ONE recip pair, DRAM
                # broadcast roundtrip, ONE [128, QB] mul.  Its ~15us chain
                # hides under the stream.
                # Blocks 4-7 (collectives 2/3, tail-critical): DEFERRED --
                # ship unnormalized O^T plus bf16 reciprocal rows through
                # the A2A (130-row slots) and scale the pins afterwards;
                # chain is ~6us, pulling the last collectives earlier.
                c0 = 2048 * b + QB * qb
                k = NQB * b + qb
                ps_o = ps_o_by_q.pop((b, qb))
                if k >= 4:
                    lrecs = []
                    for h in range(HPC):
                        nc.vector.tensor_copy(
                            out=ot[64 * h:64 * (h + 1), c0:c0 + QB],
                            in_=ps_o[h][0:64, :])
                        lsb = lbc_pool.tile([1, QB], f32, tag="lsb",
                                            name="lsb")
                        nc.vector.tensor_copy(out=lsb[:],
                                              in_=ps_o[h][64:65, :])
                        lrec32 = lbc_pool.tile([1, QB], f32, tag="lr32",
                                               name="lr32")
                        nc.vector.reciprocal_approx_fast(out=lrec32[:],
                                                         in_=lsb[:])
                        lrec = lbc_pool.tile([1, QB], bf16, tag="lrec",
                                             name="lrec")
                        nc.vector.tensor_copy(out=lrec[:], in_=lrec32[:])
                        lrecs.append(lrec)
                    a2a_block(b, qb, lrecs)
                    return
                osum = lbc_pool.tile([128, QB], f32, tag="osum", name="osum")
                ldram = dram.tile([2, QB], f32, tag="ld", name="ld")
                for h in range(HPC):
                    nc.vector.tensor_copy(out=osum[64 * h:64 * (h + 1), :],
                                          in_=ps_o[h][0:64, :])
                    lsb = lbc_pool.tile([1, QB], f32, tag="lsb", name="lsb")
                    nc.vector.tensor_copy(out=lsb[:], in_=ps_o[h][64:65, :])
                    lrec = lbc_pool.tile([1, QB], f32, tag="lrec",
                                         name="lrec")
                    nc.vector.reciprocal_approx_fast(out=lrec[:],
                                                     in_=lsb[:])
                    nc.sync.dma_start(out=ldram[h:h + 1, :], in_=lrec[:])
                lbc = lbc_pool.tile([128, QB], f32, tag="lbc", name="lbc")
                for h in range(HPC):
                    nc.sync.dma_start(
                        out=lbc[64 * h:64 * (h + 1), :],
                        in_=ldram[h:h + 1, :].to_broadcast([64, QB]))
                nc.vector.tensor_mul(out=ot[:, c0:c0 + QB], in0=osum[:],
                                     in1=lbc[:])
                a2a_block(b, qb, None)

            # ---- striped mini-AllToAll (head-space -> row-space) ----
            # q-block (b, qb) == block k = 4b + qb.  Dest core j receives
            # ot cols [2048b + 512qb + 64j, +64) from every core and owns
            # local out rows [64k, 64k+64).  Blocks are shipped in PAIRS
            # (one collective per 128 output rows): slot j = [128, 2, 64].
            from concourse import mybir as _mb

            def prime_cc():
                # tiny dummy AllToAll (garbage data, output unused) so the
                # CC stream's one-time barrier + init (~30-45us) runs
                # during the PE-bound QKV phase
                pin = dram.tile([NCORES, 1, 64], bf16, tag="a2ain",
                                name="prime_in")
                pout = dram.tile([NCORES, 1, 64], bf16, tag="a2aout",
                                 name="prime_out")
                nc.gpsimd.collective_compute(
                    "AllToAll", _mb.AluOpType.bypass, replica_groups=rg,
                    ins=[pin.opt()], outs=[pout.opt()])

            a_ins = {}
            a2a_outs = {}

            def a2a_block(b, qb, lrecs):
                # blocks ship in pairs: collective c covers blocks (2c,
                # 2c+1) == output rows [128c, 128c+128).  ONE consolidated
                # DMA per block for O^T: the j-scatter runs on the DRAM
                # side where any stride order is legal.  Deferred blocks
                # (lrecs set) use 130-row slots whose rows 128/129 carry
                # this core's per-head reciprocal denominators.
                k = NQB * b + qb
                c, idx = k // 2, k % 2
                nr = 130 if k >= 4 else 128
                if idx == 0:
                    a_ins[c] = dram.tile([NCORES, nr, 2, 64], bf16,
                                         tag="a2ain", name="a2ain")
                a_in = a_ins[c]
                base = 2048 * b + QB * qb
                nc.sync.dma_start(
                    out=a_in[:, 0:128, idx, :].rearrange("j p t -> p j t"),
                    in_=ot[:, base:base + QB].rearrange("p (j t) -> p j t",
                                                        j=NCORES))
                if lrecs is not None:
                    for h in range(HPC):
                        nc.sync.dma_start(
                            out=a_in[:, 128 + h:129 + h, idx, :].rearrange(
                                "j one t -> one j t"),
                            in_=lrecs[h][:].rearrange("one (j t) -> one j t",
                                                      j=NCORES))
                if idx == 0:
                    return
                del a_ins[c]
                a_out = dram.tile([NCORES, nr, 2, 64], bf16, tag="a2aout",
                                  name="a2aout")
                nc.gpsimd.collective_compute(
                    "AllToAll", _mb.AluOpType.bypass, replica_groups=rg,
                    ins=[a_in.opt()], outs=[a_out.opt()])
                a2a_outs[c] = a_out

            # pins: collective c covers out rows [128c, 128c+128); slice
            # [:, j, :, :] is the [128, 128] lhsT chunk for contraction
            # block j.
            pins_by_c = {}
            sc_by_c = {}

            def pins_load(c):
                # one consolidated DMA on the gpsimd queue, so the
                # wait-for-collective never head-of-line-blocks the sync
                # queue's normalize DMAs or any compute engine
                a_out = a2a_outs.pop(c)
                pinbig = pin_pool.tile([128, NCORES, 2, 64], bf16,
                                       tag="pin", name="pin")
                nc.gpsimd.dma_start(
                    out=pinbig[:],
                    in_=a_out[:, 0:128, :, :].rearrange("j p a t -> p j a t"))
                pins_by_c[c] = pinbig
                if c >= 2:
                    scbig = pin_pool.tile([128, NCORES, 2, 64], bf16,
                                          tag="pin", name="scbig")
                    for h in range(HPC):
                        nc.gpsimd.dma_start(
                            out=scbig[64 * h:64 * (h + 1), :, :, :],
                            in_=a_out[:, 128 + h:129 + h, :, :]
                            .rearrange("j one a t -> one j a t")
                            .to_broadcast([64, NCORES, 2, 64]))
                    sc_by_c[c] = scbig

            def pins_scale(c):
                # post-A2A normalization for deferred collectives; emitted
                # only at points where the VE queue is otherwise idle
                # (post-loop), so its wait on the pins DMAs is harmless
                pinbig, scbig = pins_by_c[c], sc_by_c.pop(c)
                pinS = pin_pool.tile([128, NCORES, 2, 64], bf16,
                                     tag="pin", name="pinS")
                nc.vector.tensor_mul(out=pinS[:], in0=pinbig[:],
                                     in1=scbig[:])
                pins_by_c[c] = pinS

            proj_state = {}

            def proj_half(c, n, uh):
                # collective c -> out rows [128c, 128c+128), cols [512n,
                # +512), contraction blocks j in [4uh, 4uh+4)
                pins = pins_by_c[c]
                mc, r0 = 128, 128 * c
                if (c, n) not in proj_state:
                    proj_state[(c, n)] = aux.tile([128, TB], f32, tag="aux",
                                                  name="pjps")
                ps = proj_state[(c, n)][0:mc, :]
                for j in range(4 * uh, 4 * uh + 4):
                    nc.tensor.matmul(
                        ps,
                        lhsT=pins[:, j, :, :].rearrange("p a t -> p (a t)"),
                        rhs=wproj_sb[:, j, 512 * n:512 * (n + 1)],
                        start=(j == 0),
                        stop=(j == NCORES - 1) and not with_bproj)
                if uh == 0:
                    return
                if with_bproj:
                    nc.tensor.matmul(
                        ps, lhsT=ones_col[:, 0:mc],
                        rhs=bproj_sb[:, 512 * n:512 * (n + 1)],
                        start=False, stop=True)
                del proj_state[(c, n)]
                if (c, "osb") not in proj_state:
                    proj_state[(c, "osb")] = out_pool.tile(
                        [128, D], f32, tag="osb", name="osb")
                osb = proj_state[(c, "osb")]
                nc.vector.tensor_copy(out=osb[0:mc, 512 * n:512 * (n + 1)],
                                      in_=ps)
                if n == 1:
                    del proj_state[(c, "osb")]
                    del pins_by_c[c]
                    nc.sync.dma_start(out=out[r0:r0 + mc, :],
                                      in_=osb[0:mc, :])

            def proj_all(c):
                proj_half(c, 0, 0)
                proj_half(c, 0, 1)
                proj_half(c, 1, 0)
                proj_half(c, 1, 1)

            # ---- emission order ----
            # u[tb] = [dma, qk0a, qk0b, qk1a, qk1b, v0, v1, v2, v3]
            u = {tb: qkv_tblock_units(tb) for tb in range(NTB)}
            prime_cc()
            for f in u[0][:5]:    # tb0 dma+qk only; v deferred so the
                f()               # first S^T isn't queued behind it
            fill = {}

            def put(g, *fns):
                fill.setdefault(g, []).extend(fns)

            # b0 phase: just-in-time qk for tb1-3 (kt chunk 4j needed by
            # chunk 4j), V units placed PVLAG chunks before their PV
            put(0, u[0][5])
            put(1, u[1][0], u[1][1])
            put(2, u[1][2], u[1][3])
            put(3, u[1][4], u[0][6])
            put(4, u[0][7], u[0][8])
            put(5, u[2][0], u[1][5])
            put(6, u[2][1], u[2][2])
            put(7, u[2][3], u[2][4])
            put(8, u[1][6], u[1][7])
            put(9, u[3][0], u[1][8])
            put(10, u[3][1], u[3][2])
            put(11, u[3][3], u[3][4])
            put(12, u[2][5], u[2][6])
            put(13, u[2][7], u[2][8])
            put(14, u[3][5], u[3][6])
            put(15, u[3][7], u[3][8])
            # b1 QKV: no urgency, one unit per chunk
            for i, tb in enumerate((4, 5, 6, 7)):
                g0 = 18 + 10 * i
                for k in range(9):
                    put(g0 + k, u[tb][k])
            put(40, emit_wproj_loads)
            # Collective triggers are emitted at chunks 37/69/101/133; the
            # pins loads are placed AFTER the next collective's trigger on
            # the gpsimd queue so a slow flight can never delay a trigger,
            # and proj positions carry ~30us of margin over the measured
            # land times so they never head-of-line-block the strict-FIFO
            # PE queue (a 5us PE stall also costs ~8us of HAM re-throttle).
            # proj halves are spread one per chunk so the 2-deep exp
            # pipeline never drains.
            put(71, lambda: pins_load(0))
            for i in range(4):
                put(92 + i, (lambda nn, uu: lambda: proj_half(0, nn, uu))(
                    i // 2, i % 2))
            put(103, lambda: pins_load(1))
            for i in range(4):
                put(124 + i, (lambda nn, uu: lambda: proj_half(1, nn, uu))(
                    i // 2, i % 2))

            NG = len(CHUNKS)
            for g in range(NG + PVLAG):
                if g in fill:
                    for fn in fill[g]:
                        fn()
                if g < NG:
                    emit_st(g)
                if g >= PVLAG:
                    emit_pv(g - PVLAG)

            # tail: proj for collectives 2 (pair) and 3 (block 6) fills the
            # last collective's flight window; dummy matmuls keep the PE's
            # HAM clock warm until block 7's pins arrive
            def dummy_mms(n):
                ps = aux.tile([128, TB], f32, tag="aux", name="warm")
                for i in range(n):
                    nc.tensor.matmul(
                        ps[:],
                        lhsT=wproj_sb[:, 0, 0:128],
                        rhs=wproj_sb[:, 1, 0:512],
                        start=True, stop=True)

            pins_load(2)
            pins_load(3)
            pins_scale(2)
            pins_scale(3)
            dummy_mms(4)
            proj_all(2)
            dummy_mms(40)
            proj_all(3)

    nc.finalize()
    return nc


def _prep_inputs(x, attention_mask, w_attn, b_attn, w_proj, b_proj):
    x = np.asarray(x, np.float32)
    xt = np.ascontiguousarray(x.reshape(BT, D).T).astype(BF16)
    w_attn = np.asarray(w_attn, np.float32)
    b_attn = np.asarray(b_attn, np.float32)
    wp = np.ascontiguousarray(np.asarray(w_proj, np.float32)).astype(BF16)
    scale = 1.0 / np.sqrt(HD)
    am = np.asarray(attention_mask, np.float32)
    with_mask = bool(np.any(am))
    with_battn = bool(np.any(b_attn))
    with_bproj = bool(np.any(np.asarray(b_proj)))
    mask2 = None
    if with_mask:
        m2 = (2.0 * am).reshape(B, T // 128, 128)
        mask2 = np.ascontiguousarray(m2.transpose(2, 0, 1).reshape(128, -1))
    in_maps = []
    for c in range(NCORES):
        h0, h1 = HPC * c, HPC * c + 1
        qkcols = []
        vcols = []
        for h in (h0, h1):
            qkcols.append(w_attn[:, HD * h:HD * (h + 1)] * scale)      # q
            qkcols.append(w_attn[:, D + HD * h:D + HD * (h + 1)])      # k
            vcols.append(w_attn[:, 2 * D + HD * h:2 * D + HD * (h + 1)])
        wqk = np.ascontiguousarray(np.concatenate(qkcols, axis=1)).astype(BF16)
        wv = np.ascontiguousarray(np.concatenate(vcols, axis=1)).astype(BF16)
        m = {"xt": xt, "w_qk": wqk, "w_v": wv, "w_proj": wp}
        if with_mask:
            m["mask2"] = mask2
        if with_battn:
            bqk = []
            bv = []
            for h in (h0, h1):
                bqk.append(b_attn[HD * h:HD * (h + 1)] * scale)
                bqk.append(b_attn[D + HD * h:D + HD * (h + 1)])
                bv.append(b_attn[2 * D + HD * h:2 * D + HD * (h + 1)])
            m["b_qk"] = np.ascontiguousarray(
                np.concatenate(bqk)[None, :].astype(BF16))
            m["b_v"] = np.ascontiguousarray(
                np.concatenate(bv)[None, :].astype(BF16))
        if with_bproj:
            m["b_proj"] = np.ascontiguousarray(
                np.asarray(b_proj, np.float32)[None, :].astype(BF16))
        in_maps.append(m)
    return in_maps, (with_mask, with_battn, with_bproj)


def _run(inputs, trace=False, tmpdir=None):
    from concourse.bass_utils import run_bass_kernel_spmd

    in_maps, key = _prep_inputs(**inputs)
    if key not in _CACHE:
        _CACHE[key] = _build(*key)
    nc = _CACHE[key]
    try:
        res = run_bass_kernel_spmd(nc, in_maps, core_ids=list(range(NCORES)),
                                   trace=trace, tmpdir=tmpdir)
    except Exception as e:
        if "unrecoverable" not in str(e) and "UNAVAILABLE" not in str(e):
            raise
        import ctypes
        lib = ctypes.CDLL("/opt/axon/libaxon_pjrt.so")
        if hasattr(lib, "axon_reset"):
            lib.axon_reset.restype = ctypes.c_int64
            lib.axon_reset()
        res = run_bass_kernel_spmd(nc, in_maps, core_ids=list(range(NCORES)),
                                   trace=trace, tmpdir=tmpdir)
    # local out rows: 8 chunks of 64; chunk k = 4b + qb holds global rows
    # [2048b + 512qb + 64c, +64) for core c
    y = np.empty((B, T, D), np.float32)
    for c in range(NCORES):
        o = res.results[c]["out"]
        for k in range(NBLK):
            b, qb = k // NQB, k % NQB
            r0 = QB * qb + 64 * c
            y[b, r0:r0 + 64] = o[64 * k:64 * k + 64]
    return y, res


def kernel(**inputs) -> np.ndarray:
    y, _ = _run(inputs, trace=False)
    return y
